# revision 60
# baseline (speedup 1.0000x reference)
"""Trainium2 Bass kernel for the SE + patch-correlation-attention + down-conv module.

Trace-driven redesign of the 188us baseline; measured 147-160us on HW (the
absolute number drifts +-10% run to run with host-side core-start skew).

Sharding (8 cores): core k owns the 12 image rows [12k, 12k+12).  Each core:
  1. SE pool computed LOCALLY on the PE (no SE AllGather: the baseline lost
     ~45us waiting for barrier + ncfw init + AllGather before any real work).
     The host ships the full image fp8, (position, channel)-major,
     partition-interleaved [128, 72, 256]; 36 accumulating 512-col
     ones-matmuls reduce it to the channel pool while the load streams in
     (fp8 validated in numpy: output error unchanged vs bf16; small per-core
     transfers also reduce core start skew, which every collective pays).
  2. pool row -> PE transpose -> SE gate y; S = sigmoid(x*y), FF = x*y maps
     on a bf16 14x100 padded halo slab (separate small bf16 input; wrapped
     edge rows masked via msk on BOTH maps)
  3. phase 2: 9-shift correlation products as FLAT contiguous bf16 DVE ops
     (only stride-1 single-dim APs hit the DVE 2x_1p mode; strided 3D views
     run 1x) + one-hot PE reduction over the padded flat window
     (A_ps [9, 1196], psum-bank chunks 512/512/172).  s2/ff2 are +1-element
     flat-shifted copies so dj=+-1 windows stay 4B-aligned.  ct-major order
     so products start as soon as the ct0 S map is ready.
  4. softmax: exp on ACT -> RAW exp rows broadcast immediately (DRAM
     partition-broadcast, 3 queues, d-ordered); den via ones-matmul and
     1/den folded into the acc->oat compaction instead of pre-scaling the
     weights -- keeps the PE-idle bubble under the ~3.4us HAM re-throttle
     window.  A few matmuls pinned to the d=0 broadcast bridge the bubble
     (Tile hoists dependency-free warmers out of position).
  5. phase 3: weighted sum (flat 2x products + ident PE accumulate),
     oat[ct] = acc * (1/den) on DVE, PE transposes to (q, c) tiles, DMA to
     oatF, then four strided DRAM->DRAM cc-strips (2304B runs) build the A2A
     payload as the transposes land
  6. AllToAll (bf16) -> two merged contiguous rhs loads, 256x256 down matmul,
     InstanceNorm partials -> tiny AllGather + local sum, fused
     Prelu((z-mu)*inv) on ACT, two output DMAs on separate queues
Host gathers the 8 (256, 12, 96) slices.

PSUM budget (8 banks): phases 2+3 share one pool: slotA (A -> den -> acc0,
3 banks, sequential same-tag reuse) + acc1 (3) + 2 transpose slots (1+1).
All psum tiles are padded to whole 2KB banks so matmul chunk offsets stay
bank-aligned (one matmul output must not cross a bank).

PE HAM: the clock gates to 1.2GHz after ~3.4us idle and needs ~3.4us of
sustained activity to return to 2.4GHz; warmup matmuls run before the pool.
DMA: HWDGE queues are descriptor-rate limited, so big loads use one
contiguous per-partition run per DMA.

A dummy 64B AllGather is the first instruction: the first collective of an
execution pays a ~30-60us one-time barrier+ncfw init which would otherwise
serialize in front of the AllToAll.

A2A scramble (proven in the baseline): receiver j needs
rhs[c'=32k+cc, s*36+t] = out_k[q=36*cc+t, c=32*j+s].  Sender k writes
a2a_in[j][cc, s, t] = oat_ct[32a+s, 36cc+t] (j = 4ct+a), so a2a_out[k] drops
straight into rhs partitions [32k, 32k+32).
"""
import numpy as np

C, H, W, M = 256, 96, 96, 8
RPC = H // M          # 12 rows per core
P = RPC * W           # 1152 positions per core
SLAB = RPC + 2        # 14 rows incl. halo
WP = 100              # padded slab width (even stride, image col j at col j+2)
FLAT = SLAB * WP      # 1400
F0 = WP + 2           # flat offset of (row 1, col 2) = first valid center
VF = (SLAB - 3) * WP + W  # 1196: valid flat window (rows 1..12, incl row pads)
HW = H * W            # 9216
VFP = 1200            # padded tile width for VF-indexed tensors (12 * WP)
SHIFTS = [(di, dj) for di in (-1, 0, 1) for dj in (-1, 0, 1)]
ACHUNKS = [(0, 512), (512, 512), (1024, VF - 1024)]   # A/acc psum chunks
ZCHUNKS = [(0, 512), (512, 512), (1024, 128)]         # down-matmul chunks

_cache = {}


def _build(sim_safe=False):
    import concourse.bass as bass
    from concourse import bacc
    import concourse.mybir as mybir
    from concourse.tile import TileContext
    from concourse.masks import make_identity

    fp32 = mybir.dt.float32
    bf16 = mybir.dt.bfloat16
    AF = mybir.ActivationFunctionType
    Alu = mybir.AluOpType
    GROUPS = [list(range(M))]

    nc = bacc.Bacc()

    fp8 = mybir.dt.float8e4
    # xqT: full image in fp8, (position, channel)-major, partition-interleaved
    # host layout [128, 72, 256]: tile t = positions [128t, 128t+128)
    xqT = nc.declare_dram_parameter("xqT", [128, 72, C], fp8, isOutput=False)
    xs = nc.declare_dram_parameter("xs", [C, SLAB, W], bf16, isOutput=False)
    msk = nc.declare_dram_parameter("msk", [128, 2], fp32, isOutput=False)
    w1t = nc.declare_dram_parameter("w1t", [C, 16], fp32, isOutput=False)
    b1 = nc.declare_dram_parameter("b1", [16, 1], fp32, isOutput=False)
    w2t = nc.declare_dram_parameter("w2t", [16, C], fp32, isOutput=False)
    b2 = nc.declare_dram_parameter("b2", [C, 1], fp32, isOutput=False)
    dwt = nc.declare_dram_parameter("dwt", [C, C], bf16, isOutput=False)
    outp = nc.declare_dram_parameter("out", [C, RPC, W], bf16, isOutput=True)

    dumm_in = nc.dram_tensor("dumm_in", [4, 4], fp32)
    dumm_out = nc.dram_tensor("dumm_out", [32, 4], fp32, addr_space="Shared")
    a2a_in = nc.dram_tensor("a2a_in", [M, 32, P], bf16)
    a2a_out = nc.dram_tensor("a2a_out", [M, 32, P], bf16)
    st_part = nc.dram_tensor("st_part", [128, 4], fp32)
    st_gath = nc.dram_tensor("st_gath", [M, 128, 4], fp32, addr_space="Shared")

    e_dram = nc.dram_tensor("e_dram", [9, VFP], bf16)   # raw exp rows
    r_dram = nc.dram_tensor("r_dram", [1, VFP], fp32)   # 1/den row
    # attention output in flat (q, c)-major layout; a2a_in[j][cc] is the
    # contiguous slice oatF[cc*9216 + 1152*j : +1152]
    oatF = nc.dram_tensor("oatF", [P * C], bf16)
    dma = nc.default_dma_engine

    with TileContext(nc) as tc:
        with (
            tc.tile_pool(name="const", bufs=1) as cp,
            tc.tile_pool(name="sb", bufs=1) as sp,
            tc.tile_pool(name="work", bufs=8) as wp,
        ):
            # ---------- dummy collective: pays the one-time barrier+ncfw init
            nc.gpsimd.collective_compute(
                "AllGather", Alu.bypass, replica_groups=GROUPS,
                ins=[dumm_in[:, :]], outs=[dumm_out[:, :]],
            )

            eps_sb = cp.tile([128, 1], fp32)
            nc.vector.memset(eps_sb, 1e-5)

            # ---------- constants ----------
            ident = cp.tile([128, 128], bf16)
            make_identity(nc, ident)
            # one-hot selector columns: e_all[:, d, m] = (m == d), used as lhsT
            # so shift d's channel-reduction lands in row d of the (9, VF) psum
            e_all = cp.tile([128, 9, 9], bf16)
            nc.vector.memset(e_all, 0.0)
            for d in range(9):
                nc.vector.memset(e_all[:, d, d : d + 1], 1.0)
            ones_99 = cp.tile([9, 9], bf16)
            nc.vector.memset(ones_99, 1.0)
            warm_src = cp.tile([128, 512], bf16)
            nc.vector.memset(warm_src, 0.0)
            ones_col = cp.tile([128, 1], fp8)
            nc.vector.memset(ones_col, 1.0)
            one_11 = cp.tile([1, 1], fp32)
            nc.vector.memset(one_11, 1.0)

            # ---------- front: fp8 transposed pool image + bf16 slab, loads
            # first on all 3 HWDGE queues, weight DMAs behind them -----------
            # (fp8 keeps the per-core host transfer small: big transfers skew
            # core start times, and every collective waits for the last core)
            xt_sb = sp.tile([128, 72, C], fp8, tag="xt")
            xs_sb = [sp.tile([128, SLAB, W], bf16, tag=f"xs{ct}", name=f"xs{ct}") for ct in range(2)]
            qs = [dma, nc.scalar, nc.gpsimd]
            # one contiguous per-partition run per DMA (descriptor-rate, not
            # bandwidth, limits the HWDGE queues); 12-tile chunks so the pool
            # matmuls start on the first arrival
            for ch in range(6):
                qs[ch % 3].dma_start(
                    out=xt_sb[:, 12 * ch : 12 * ch + 12, :],
                    in_=xqT[:, 12 * ch : 12 * ch + 12, :],
                )
            for ct in range(2):
                qs[ct].dma_start(
                    out=xs_sb[ct],
                    in_=xs[128 * ct : 128 * ct + 128, :, :],
                )

            msk_sb = cp.tile([128, 2], fp32)
            dma.dma_start(out=msk_sb, in_=msk[:, :])
            b1_sb = cp.tile([16, 1], fp32)
            nc.scalar.dma_start(out=b1_sb, in_=b1[:, :])
            b2_sb = cp.tile([128, 2], fp32)
            w1_sb = [cp.tile([128, 16], fp32, tag=f"w1_{ct}", name=f"w1_{ct}") for ct in range(2)]
            dw_sb = [cp.tile([128, C], bf16, tag=f"dw_{ct}", name=f"dw_{ct}") for ct in range(2)]
            for ct in range(2):
                nc.scalar.dma_start(out=b2_sb[:, ct : ct + 1], in_=b2[128 * ct : 128 * ct + 128, :])
                dma.dma_start(out=w1_sb[ct], in_=w1t[128 * ct : 128 * ct + 128, :])
                nc.gpsimd.dma_start(out=dw_sb[ct], in_=dwt[128 * ct : 128 * ct + 128, :])
            w2_sb = cp.tile([16, C], fp32)
            dma.dma_start(out=w2_sb, in_=w2t[:, :])

            # ---------- SE pool on the (otherwise idle) PE: ones-reduction
            # over position tiles, 1024-col accumulating matmuls -------------
            with tc.tile_pool(name="ps_front", bufs=1, space="PSUM") as pf:
                warm_ps = pf.tile([128, 512], fp32, tag="warm")
                for i in range(14):
                    nc.tensor.matmul(warm_ps, ident, warm_src, start=True, stop=True)
                pool_ps = pf.tile([1, 512], fp32, tag="poolps")
                for g in range(36):
                    nc.tensor.matmul(
                        pool_ps, ones_col,
                        xt_sb[:, 2 * g : 2 * g + 2, :].rearrange("p t c -> p (t c)"),
                        start=(g == 0), stop=(g == 35),
                    )
                # fold the 2 position-groups, then transpose the channel row
                # onto partitions for the gate matmul
                pool_row = sp.tile([1, C], fp32, tag="poolrow")
                nc.vector.tensor_reduce(
                    out=pool_row,
                    in_=pool_ps.rearrange("p (t c) -> p c t", c=C),
                    axis=mybir.AxisListType.X, op=Alu.add,
                )
                tp_po = pf.tile([128, 1024], fp32, tag="tppo")
                ssum_sb = sp.tile([128, 2], fp32, tag="sesum")
                for ct in range(2):
                    # transpose the [1, 128] channel row onto partitions:
                    # out = row.T @ [[1]]
                    nc.tensor.matmul(
                        tp_po[:, ct : ct + 1], pool_row[:, 128 * ct : 128 * ct + 128],
                        one_11, start=True, stop=True,
                    )
                    nc.vector.tensor_copy(out=ssum_sb[:, ct : ct + 1], in_=tp_po[:, ct : ct + 1])

                # ---------- SE gate ----------
                h_ps = pf.tile([16, 512], fp32, tag="hps")
                for ct in range(2):
                    nc.tensor.matmul(
                        h_ps[:, 0:1], w1_sb[ct], ssum_sb[:, ct : ct + 1],
                        start=(ct == 0), stop=(ct == 1),
                    )
                h_sb = sp.tile([16, 1], fp32)
                nc.vector.tensor_scalar(
                    out=h_sb, in0=h_ps[:, 0:1], scalar1=1.0 / HW, scalar2=b1_sb[:, 0:1],
                    op0=Alu.mult, op1=Alu.add,
                )
                nc.vector.tensor_scalar(
                    out=h_sb, in0=h_sb, scalar1=0.0, scalar2=None, op0=Alu.max,
                )
                y_ps = pf.tile([128, 512], fp32, tag="yps")
                y_sb = sp.tile([128, 2], fp32, tag="ygate")
                for ct in range(2):
                    nc.tensor.matmul(
                        y_ps[:, ct : ct + 1], w2_sb[:, 128 * ct : 128 * ct + 128], h_sb,
                        start=True, stop=True,
                    )
                    nc.scalar.activation(out=y_sb[:, ct : ct + 1], in_=y_ps[:, ct : ct + 1],
                                         func=AF.Sigmoid, bias=b2_sb[:, ct : ct + 1], scale=1.0)

            # ---------- FF and S maps (bf16, zero-padded 14x100 layout) ------
            # flat layout: row r at [100r, 100r+100), image col j at 100r+2+j.
            # s2/ff2 are the maps shifted +1 flat element so every dj=+-1
            # shifted window starts at an EVEN element offset (DVE 2x needs
            # 4B alignment).
            ff_sb = [sp.tile([128, SLAB, WP], bf16, tag=f"ff{ct}", name=f"ff{ct}") for ct in range(2)]
            s_sb = [sp.tile([128, SLAB, WP], bf16, tag=f"s{ct}", name=f"s{ct}") for ct in range(2)]
            ff2_sb = [sp.tile([128, FLAT], bf16, tag=f"ff2{ct}", name=f"ff2{ct}") for ct in range(2)]
            s2_sb = [sp.tile([128, FLAT], bf16, tag=f"s2{ct}", name=f"s2{ct}") for ct in range(2)]

            def flat(t):
                return t.rearrange("c r w -> c (r w)")

            # pad columns zeroed once; the flat-shifted s2/ff2 copies inherit
            for ct in range(2):
                for t in (s_sb[ct], ff_sb[ct]):
                    nc.vector.memset(t[:, :, 0:2], 0.0)
                    nc.vector.memset(t[:, :, 98:100], 0.0)

            def mask_rows(t, eng):
                # zero invalid halo rows (top/bottom image edge)
                eng.tensor_scalar(
                    out=t[:, 0, 2:98], in0=t[:, 0, 2:98],
                    scalar1=msk_sb[:, 0:1], scalar2=None, op0=Alu.mult,
                )
                eng.tensor_scalar(
                    out=t[:, 13, 2:98], in0=t[:, 13, 2:98],
                    scalar1=msk_sb[:, 1:2], scalar2=None, op0=Alu.mult,
                )

            # S maps first (phase 2 only needs S); FF afterwards on the same
            # ACT queue — phase 3 consumes it much later
            for ct in range(2):
                nc.scalar.activation(
                    out=s_sb[ct][:, :, 2:98], in_=xs_sb[ct],
                    func=AF.Sigmoid, scale=y_sb[:, ct : ct + 1],
                )
                mask_rows(s_sb[ct], nc.vector)
                nc.vector.tensor_copy(out=s2_sb[ct][:, 1:FLAT], in_=flat(s_sb[ct])[:, 0 : FLAT - 1])
            for ct in range(2):
                nc.scalar.activation(
                    out=ff_sb[ct][:, :, 2:98], in_=xs_sb[ct],
                    func=AF.Copy, scale=y_sb[:, ct : ct + 1],
                )
                mask_rows(ff_sb[ct], nc.gpsimd)
                nc.gpsimd.tensor_copy(out=ff2_sb[ct][:, 1:FLAT], in_=flat(ff_sb[ct])[:, 0 : FLAT - 1])

            # prewarm the ACT function tables on the idle ACT queue (each
            # first-use otherwise pays a ~1.3us table load on the critical
            # path: Exp in the softmax bubble, Sqrt/Square in the tail)
            tw_scr = cp.tile([128, 1], fp32)
            for fn in (AF.Exp, AF.Sqrt, AF.Square):
                nc.scalar.activation(out=tw_scr, in_=eps_sb, func=fn)

            def svf(ct, di, dj):
                # flat [128, VF] view of the shifted sigmoid map, even offsets
                if dj == 0:
                    return flat(s_sb[ct])[:, F0 + WP * di : F0 + WP * di + VF]
                return s2_sb[ct][:, F0 + WP * di + dj + 1 : F0 + WP * di + dj + 1 + VF]

            def ffvf(ct, di, dj):
                if dj == 0:
                    return flat(ff_sb[ct])[:, F0 + WP * di : F0 + WP * di + VF]
                return ff2_sb[ct][:, F0 + WP * di + dj + 1 : F0 + WP * di + dj + 1 + VF]

            exp_sb = sp.tile([9, VFP], bf16, tag="exps")
            rec_sb = sp.tile([1, VFP], fp32, tag="recs")
            rec_bc = sp.tile([128, VFP], fp32, tag="recbc")
            rep_sb = [sp.tile([128, VFP], bf16, tag=f"rep{d}", name=f"rep{d}") for d in range(9)]
            oat = [sp.tile([128, P], bf16, tag=f"oat{ct}", name=f"oat{ct}") for ct in range(2)]
            tq = [sp.tile([128, C], bf16, tag=f"tq{tau}", name=f"tq{tau}") for tau in range(9)]
            # exp pad columns must be zero (the den matmul streams them and
            # the rep broadcasts replicate them against shifted FF data)
            nc.vector.memset(
                exp_sb.rearrange("p (b f) -> p b f", f=WP)[:, :, 96:100], 0.0)

            # strided [*, 12, 96] valid-position view of a [*, >=VFP] tile
            def valid(t):
                return t[:, 0:VFP].rearrange("c (b f) -> c b f", f=WP)[:, :, 0:96]

            # psum pool for phases 2+3: slotA (A -> den -> acc0), acc1, tp x2
            # = 3+3+1+1 banks = 8.  All tiles sized to whole psum banks.
            with tc.tile_pool(name="ps_main", bufs=1, space="PSUM") as pm:
                # ---------- phase 2: correlation  A[d, p] = sum_c S*S_d ------
                slot_a = pm.tile([128, 1536], fp32, tag="slotA", name="A")
                A_ps = slot_a[0:9, :]
                # ct-major: all ct0 products first so phase 2 starts as soon
                # as the ct0 S map is ready
                for ct in range(2):
                    for d, (di, dj) in enumerate(SHIFTS):
                        prod = wp.tile([128, VFP], bf16, tag="prod")
                        nc.vector.tensor_tensor(
                            out=prod[:, 0:VF],
                            in0=flat(s_sb[ct])[:, F0 : F0 + VF],
                            in1=svf(ct, di, dj),
                            op=Alu.mult,
                        )
                        for (o, n) in ACHUNKS:
                            nc.tensor.matmul(
                                A_ps[:, o : o + n], e_all[:, d, :],
                                prod[:, o : o + n],
                                start=(d == 0 and ct == 0), stop=(d == 8 and ct == 1),
                            )

                # ---------- softmax: exp -> broadcast raw exp rows; 1/den is
                # computed in parallel and folded into the acc->oat copy
                nc.scalar.activation(out=valid(exp_sb), in_=valid(A_ps),
                                     func=AF.Exp, scale=1.0 / C)
                dma.dma_start(out=e_dram[:, :], in_=exp_sb)
                # broadcast queues in d-order: sync + gpsimd + ACT (the ACT
                # queue is free once exp has issued)
                bq = [dma, nc.gpsimd, nc.scalar]
                for d in range(9):
                    bq[d % 3].dma_start(
                        out=rep_sb[d],
                        in_=e_dram[d, :].partition_broadcast(128),
                    )
                slot_d = pm.tile([128, 1536], fp32, tag="slotA", name="den")
                den_ps = slot_d[0:9, :]
                for (o, n) in ACHUNKS:
                    nc.tensor.matmul(den_ps[:, o : o + n], ones_99,
                                     exp_sb[:, o : o + n], start=True, stop=True)
                nc.vector.reciprocal_approx_fast(out=rec_sb, in_=den_ps[0:1, 0:VFP])
                dma.dma_start(out=r_dram[:, :], in_=rec_sb)
                nc.scalar.dma_start(
                    out=rec_bc, in_=r_dram[0, :].partition_broadcast(128))

                # ---------- phase 3: out[c,p] = (sum_d exp_d * FF_d) / den ---
                acc = [None, None]
                acc[0] = pm.tile([128, 1536], fp32, tag="slotA", name="acc0")
                acc[1] = pm.tile([128, 1536], fp32, tag="acc1", name="acc1")
                tp_ps = [pm.tile([128, 1024], bf16, tag=f"tp{i}", name=f"tp{i}") for i in range(2)]
                # bubble warmers: reading the d=0 broadcast pins them into the
                # softmax window (Tile would otherwise hoist them earlier) so
                # the PE stays at full clock into phase 3
                for i in range(4):
                    nc.tensor.matmul(acc[1][:, 0:512], ident, rep_sb[0][:, 0:512],
                                     start=True, stop=True)
                for ct in range(2):
                    for d, (di, dj) in enumerate(SHIFTS):
                        prod = wp.tile([128, VFP], bf16, tag="prod")
                        nc.vector.tensor_tensor(
                            out=prod[:, 0:VF],
                            in0=ffvf(ct, di, dj),
                            in1=rep_sb[d][:, 0:VF],
                            op=Alu.mult,
                        )
                        for (o, n) in ACHUNKS:
                            nc.tensor.matmul(
                                acc[ct][:, o : o + n], ident,
                                prod[:, o : o + n],
                                start=(d == 0), stop=(d == 8),
                            )
                    # compact to (c, q) bf16 with the softmax denominator
                    # folded in: oat = acc * (1/den)  (DVE only: gpsimd
                    # cannot read PSUM, ACT has no two-tensor op)
                    nc.vector.tensor_tensor(
                        out=oat[ct].rearrange("c (b f) -> c b f", f=96, b=12),
                        in0=valid(acc[ct]), in1=valid(rec_bc), op=Alu.mult,
                    )
                    # transpose to the flat (q, c) layout on the PE
                    for tau in range(9):
                        tpt = tp_ps[tau % 2]
                        nc.tensor.transpose(
                            tpt[:, 0:128], oat[ct][:, 128 * tau : 128 * tau + 128], ident,
                        )
                        ceng = nc.vector.tensor_copy if tau % 2 else nc.scalar.copy
                        ceng(out=tq[tau][:, 128 * ct : 128 * ct + 128], in_=tpt[:, 0:128])
                        if ct == 1:
                            deng = (dma, nc.scalar, nc.gpsimd)[tau % 3]
                            deng.dma_start(
                                out=oatF[32768 * tau : 32768 * tau + 32768].rearrange(
                                    "(q c) -> q c", c=C
                                ),
                                in_=tq[tau],
                            )

            # strided DRAM->DRAM builds the A2A payload (2304B runs):
            # a2a_in[j, cc, u] = oatF[cc*9216 + j*1152 + u].  Four cc-strips
            # so each fires as soon as its tq tiles have landed (strip s
            # covers oatF q-rows [288s, 288s+288) = tq tiles 2.25s..2.25s+2.25)
            for s in range(4):
                qs[s % 3].dma_start(
                    out=a2a_in[:, 8 * s : 8 * s + 8, :],
                    in_=oatF[73728 * s : 73728 * (s + 1)].rearrange(
                        "(cc j u) -> j cc u", cc=8, j=M),
                )

            # ---------- AllToAll ----------
            nc.gpsimd.collective_compute(
                "AllToAll", Alu.bypass, replica_groups=GROUPS,
                ins=[a2a_in[:, :, :]], outs=[a2a_out[:, :, :]],
            )

            # ---------- tail: down matmul + InstanceNorm + LeakyReLU --------
            stat_sb = sp.tile([128, 4], fp32, tag="stat")
            sq_scr = sp.tile([128, P], fp32, tag="sqscr")
            zo_sb = [sp.tile([128, P], bf16, tag=f"zo{mt}", name=f"zo{mt}") for mt in range(2)]
            rhs_sb = [sp.tile([128, P], bf16, tag=f"rhs{kt}", name=f"rhs{kt}") for kt in range(2)]
            with tc.tile_pool(name="ps_z", bufs=1, space="PSUM") as pz:
                # two merged rhs loads (4 contiguous a2a_out chunks each)
                for kt in range(2):
                    eng = (dma, nc.scalar)[kt]
                    eng.dma_start(
                        out=rhs_sb[kt],
                        in_=a2a_out[4 * kt : 4 * kt + 4].rearrange(
                            "a s p -> (a s) p"),
                    )

                z_ps = [pz.tile([128, 1536], fp32, tag=f"z{mt}", name=f"z{mt}") for mt in range(2)]
                for (o, n) in ZCHUNKS:
                    for mt in range(2):
                        for kt in range(2):
                            nc.tensor.matmul(
                                z_ps[mt][:, o : o + n],
                                dw_sb[kt][:, 128 * mt : 128 * mt + 128],
                                rhs_sb[kt][:, o : o + n],
                                start=(kt == 0), stop=(kt == 1),
                            )
                for mt in range(2):
                    # IN stats partials
                    nc.vector.tensor_reduce(
                        out=stat_sb[:, mt : mt + 1], in_=z_ps[mt][:, 0:P],
                        axis=mybir.AxisListType.X, op=Alu.add,
                    )
                    nc.scalar.activation(
                        out=sq_scr, in_=z_ps[mt][:, 0:P], func=AF.Square,
                        accum_out=stat_sb[:, 2 + mt : 3 + mt],
                    )
                dma.dma_start(out=st_part[:, :], in_=stat_sb)
                nc.gpsimd.collective_compute(
                    "AllGather", Alu.bypass, replica_groups=GROUPS,
                    ins=[st_part[:, :]], outs=[st_gath[:, :, :]],
                )
                # one-DMA readback (DRAM-side rearrange); sync HWDGE only:
                # keeps the ACT queue free so the sqrt table-load runs while
                # the collective is still in flight
                stat_all = sp.tile([128, M, 4], fp32, tag="statall")
                dma.dma_start(
                    out=stat_all, in_=st_gath.rearrange("r p four -> p r four"),
                )
                gl_sb = sp.tile([128, 4], fp32, tag="glstat")
                nc.vector.tensor_reduce(
                    out=gl_sb, in_=stat_all.rearrange("p r four -> p four r"),
                    axis=mybir.AxisListType.X, op=Alu.add,
                )

                # mu = sum/HW ; var = sumsq/HW - mu^2 ; inv = 1/sqrt(var+eps)
                ins_sb = sp.tile([128, 8], fp32, tag="instat")
                mu2 = ins_sb[:, 0:2]
                e22 = ins_sb[:, 2:4]
                inv2 = ins_sb[:, 4:6]
                nmi2 = ins_sb[:, 6:8]
                nc.vector.tensor_scalar(out=mu2, in0=gl_sb[:, 0:2],
                                        scalar1=1.0 / HW, scalar2=None, op0=Alu.mult)
                nc.vector.tensor_scalar(out=e22, in0=gl_sb[:, 2:4],
                                        scalar1=1.0 / HW, scalar2=None, op0=Alu.mult)
                nc.vector.tensor_tensor(out=inv2, in0=mu2, in1=mu2, op=Alu.mult)
                nc.vector.tensor_tensor(out=e22, in0=e22, in1=inv2, op=Alu.subtract)
                nc.scalar.activation(out=e22, in_=e22, func=AF.Sqrt, bias=eps_sb, scale=1.0)
                nc.vector.reciprocal(out=inv2, in_=e22)
                # nmi = -mu * inv  (bias for the fused Prelu normalize)
                nc.vector.scalar_tensor_tensor(out=nmi2, in0=mu2, scalar=-1.0,
                                               in1=inv2, op0=Alu.mult, op1=Alu.mult)
                # LeakyReLU((z - mu) * inv) fused on ScalarE:
                #   prelu(z*inv + (-mu*inv), alpha=0.2)
                # 4 half-tiles so the first output DMA starts after ~0.7us of
                # normalize instead of waiting for a full 1152-col pass
                oq = [dma, nc.gpsimd, nc.scalar, dma]
                for mt in range(2):
                    for hh in range(2):
                        o0 = 576 * hh
                        nc.scalar.activation(
                            out=zo_sb[mt][:, o0 : o0 + 576],
                            in_=z_ps[mt][:, o0 : o0 + 576],
                            func=AF.Copy if sim_safe else AF.Prelu,
                            bias=0.0 if sim_safe else ins_sb[:, 6 + mt : 7 + mt],
                            scale=ins_sb[:, 4 + mt : 5 + mt], alpha=0.2,
                        )
                        oq[2 * mt + hh].dma_start(
                            out=outp[128 * mt : 128 * mt + 128, 6 * hh : 6 * hh + 6, :],
                            in_=zo_sb[mt][:, o0 : o0 + 576].rearrange(
                                "c (r w) -> c r w", w=W),
                        )
    nc.compile()
    return nc


def _get_nc():
    if "nc" not in _cache:
        _cache["nc"] = _build()
    return _cache["nc"]


def _shard_inputs(x, se_w1, se_b1, se_w2, se_b2, down_w):
    import ml_dtypes

    x = np.ascontiguousarray(np.asarray(x, np.float32))[0]          # (C, H, W)
    # pool image: fp8, (position, channel)-major, partition-interleaved
    # [128, 72, 256]; the pool is roll-invariant so all cores share it
    xqT = np.ascontiguousarray(
        x.transpose(1, 2, 0).reshape(72, 128, C).transpose(1, 0, 2)
    ).astype(ml_dtypes.float8_e4m3)
    xb = x.astype(ml_dtypes.bfloat16)
    w1t = np.ascontiguousarray(np.asarray(se_w1, np.float32).T)     # (C, 16)
    b1 = np.ascontiguousarray(np.asarray(se_b1, np.float32)[:, None])
    w2t = np.ascontiguousarray(np.asarray(se_w2, np.float32).T)     # (16, C)
    b2 = np.ascontiguousarray(np.asarray(se_b2, np.float32)[:, None])
    dwt = np.ascontiguousarray(
        np.asarray(down_w, np.float32).T.astype(ml_dtypes.bfloat16)
    )                                                               # (C, C) bf16

    in_maps = []
    for k in range(M):
        # bf16 halo slab: image rows 12k-1 .. 12k+12 (wrapped; the wrapped
        # edge rows are masked on device via msk)
        lo = RPC * k - 1
        idx = (np.arange(lo, lo + SLAB)) % H
        xsk = np.ascontiguousarray(xb[:, idx, :])
        msk = np.ones((128, 2), np.float32)
        if k == 0:
            msk[:, 0] = 0.0
        if k == M - 1:
            msk[:, 1] = 0.0
        in_maps.append({
            "xqT": xqT, "xs": xsk, "msk": msk, "w1t": w1t, "b1": b1,
            "w2t": w2t, "b2": b2, "dwt": dwt,
        })
    return in_maps


def _gather(results):
    # core k's output is the contiguous 12-row slab [12k, 12k+12) of the image
    R = np.stack([np.asarray(r["out"], np.float32) for r in results])  # (8, 256, 12, 96)
    return np.ascontiguousarray(
        R.transpose(1, 0, 2, 3).reshape(1, C, H, W).astype(np.float32)
    )


def kernel(x, se_w1, se_b1, se_w2, se_b2, down_w, _trace=False):
    from concourse.bass_utils import run_bass_kernel_spmd

    nc = _get_nc()
    in_maps = _shard_inputs(x, se_w1, se_b1, se_w2, se_b2, down_w)
    res = run_bass_kernel_spmd(nc, in_maps, core_ids=list(range(M)), trace=_trace)
    out = _gather(res.results)
    if _trace:
        kernel.last_results = res
    return out
